# revision 1
# baseline (speedup 1.0000x reference)
"""Trainium2 Bass kernel for nn_Bottleneck_refine (grouped bottleneck + block mask).

Reference computation (per image b):
    m   = upsample(mask[b])            # [4,7,7] -> per-group 56x56 {0,1}
    t1  = conv1x1_g4(x * m1)           # 512 -> 128, but 1x1 commutes with mask
    a1  = m . relu(s1*t1 + c1)
    t2  = conv3x3_g4(a1)               # 128 -> 128 (pad 1)
    a2  = m . relu(s2*t2 + c2)
    y   = relu(s3*conv1x1_g4(a2) + c3 + x)

Identity used: for m in {0,1}:  m*relu(z) == relu(m*z), and the 1x1 conv
commutes with per-pixel masking, so the input mask multiply is absorbed.

Sharding: data-parallel over batch, 2 images per core on 8 cores.
Per-core HBM traffic ~= 12.9 MB in + 12.9 MB out (memory bound ~72us).

Layouts per image (all SBUF, f32, [partition, free]):
  x_g     [128, 3136]  per group g (channel-major, pixel row-major)
  a1h     [128, 58*58] halo'd masked mid activation (channels 4g x 32)
  a2s     [128, 392] per (g, superchunk): partition 32j+co = chunk 4k+j, ch co
  chunks: 7 image rows (392 px), 8 chunks, 2 superchunks of 4 chunks.

PE mapping:
  conv1: 128x32 column tiling, tile (0, 32g), one PSUM bank, channel-major out.
  conv2: 32x32 16-tile packing, tile (32g, 32j): row=group, col=chunk-in-sc.
         9 taps accumulate into bank g; output chunk-scrambled.
  conv3: 32x128 row tiling, tile (32j, 0) per (group, chunk), full-width out.
"""

import numpy as np

import concourse.bass as bass
import concourse.tile as tile
from concourse import bacc, mybir
from concourse.bass_utils import run_bass_kernel_spmd

F32 = mybir.dt.float32
BF16 = mybir.dt.bfloat16
EPS = 1e-5

N_CORES = 8
B_TOT = 16
B = B_TOT // N_CORES  # images per core
G = 4
CIN = 512
MID = 128
H = W = 56
PIX = H * W  # 3136
R = 7  # image rows per chunk
CH = R * W  # 392 pixels per chunk
NCH = H // R  # 8 chunks
SC = 4  # chunks per superchunk
NSC = NCH // SC  # 2 superchunks
HH = H + 2  # halo'd height/width (58)


def build_nc():
    # Bacc (not Bass): its compile()/finalize() pipeline legalizes sync waits
    # (>=2 waits per instruction are split into EventSemaphore instructions,
    # which this walrus build requires) and moves matmul waits to ldweights.
    nc = bacc.Bacc(None, target_bir_lowering=False)

    xs = nc.dram_tensor("xs", [B, CIN, PIX], F32, kind="ExternalInput")
    mup = nc.dram_tensor("mup", [B, G, PIX], BF16, kind="ExternalInput")
    mupS = nc.dram_tensor("mupS", [B, SC, G * NSC * CH], BF16, kind="ExternalInput")
    w1l = nc.dram_tensor("w1l", [128, G, 32], F32, kind="ExternalInput")
    w2l = nc.dram_tensor("w2l", [128, 9, 32], F32, kind="ExternalInput")
    w3l = nc.dram_tensor("w3l", [128, G, 128], F32, kind="ExternalInput")
    b1d = nc.dram_tensor("b1d", [128, 1], F32, kind="ExternalInput")
    b2d = nc.dram_tensor("b2d", [128, G], F32, kind="ExternalInput")
    b3d = nc.dram_tensor("b3d", [128, G], F32, kind="ExternalInput")
    ys = nc.dram_tensor("ys", [B, CIN, PIX], F32, kind="ExternalOutput")

    with tile.TileContext(nc) as tc:
        with (
            tc.tile_pool(name="consts", bufs=1) as consts,
            tc.tile_pool(name="xpool", bufs=12) as xpool,
            tc.tile_pool(name="mpool", bufs=2) as mpool,
            tc.tile_pool(name="a1pool", bufs=1) as a1pool,
            tc.tile_pool(name="a2pool", bufs=8) as a2pool,
            tc.tile_pool(name="upool", bufs=4) as upool,
            tc.tile_pool(name="opool", bufs=5) as opool,
            tc.tile_pool(name="psum", bufs=1, space="PSUM") as psum,
        ):
            # ---- constants (loaded once) ----
            w1sb = consts.tile([128, G, 32], F32)
            w2sb = consts.tile([128, 9, 32], F32)
            w3sb = consts.tile([128, G, 128], F32)
            b1sb = consts.tile([128, 1], F32)
            b2sb = consts.tile([128, G], F32)
            b3sb = consts.tile([128, G], F32)
            nc.sync.dma_start(out=w1sb, in_=w1l[:])
            nc.sync.dma_start(out=w2sb, in_=w2l[:])
            nc.sync.dma_start(out=w3sb, in_=w3l[:])
            nc.sync.dma_start(out=b1sb, in_=b1d[:])
            nc.sync.dma_start(out=b2sb, in_=b2d[:])
            nc.sync.dma_start(out=b3sb, in_=b3d[:])

            # PSUM bank plan (8 banks, one tag per bank, bufs=1 each):
            #   stage A (conv1): banks 0-3 rotate per chunk
            #   stage B (conv2): banks 4-7 held per superchunk (bank = group)
            #   stage C (conv3): banks 0-3 (bank = row tile j), 4-way concurrency
            def pbank(i, name):
                return psum.tile([128, 512], F32, name=name, tag=f"bk{i}")[:, :CH]

            SCW = SC * CH  # pixels per superchunk (1568)

            # PE warmup: keep TensorE busy during the input-DMA head so the
            # HAM clock gate reaches 8/8 before conv1; results are discarded.
            warm = psum.tile([128, 512], F32, name="warm", tag="bk7")[:, :128]
            for wi in range(24):
                nc.tensor.matmul(
                    warm[0:32, :],
                    w1sb[:, 0, :],
                    w3sb[:, 0, :],
                    start=True,
                    stop=True,
                    tile_position=(0, 0),
                )

            for b in range(B):
                # ---- load x per (group, superchunk-half): compute starts early
                xg = {}
                for k in range(NSC):
                    for g in range(G):
                        xt = xpool.tile([128, SCW], F32, name=f"x_{b}_{g}_{k}", tag="x")
                        eng = nc.sync if g % 2 == 0 else nc.scalar
                        nh = 2 if k == 0 else 1  # halve head loads: compute starts earlier
                        hw = SCW // nh
                        for h2 in range(nh):
                            eng.dma_start(
                                out=xt[:, h2 * hw : (h2 + 1) * hw],
                                in_=xs[
                                    b,
                                    128 * g : 128 * (g + 1),
                                    SCW * k + h2 * hw : SCW * k + (h2 + 1) * hw,
                                ],
                            )
                        xg[(g, k)] = xt

                # ---- masks via broadcast DMA (partition step-0) ----
                # channel-major: partition 32g+c <- mup[b, g, :]; split per half
                mM = mpool.tile([128, PIX], BF16, name=f"mM_{b}", tag="mM")
                stgM = mpool.tile([128, PIX], BF16, name=f"stgM_{b}", tag="stgM")
                nc.gpsimd.memset(stgM, 0.0)
                for g in range(G):
                    nc.gpsimd.dma_start(
                        out=stgM[32 * g : 32 * g + 1, :], in_=mup[b, g : g + 1, :]
                    )
                nc.vector.stream_shuffle(mM, stgM, [0] * 32)

                # ---- halo'd a1 (zeroed borders via full memset) ----
                a1h = a1pool.tile([128, HH, HH], F32, name=f"a1h_{b}", tag="a1h")
                nc.gpsimd.memset(a1h, 0.0)

                # ---- stage A: conv1 + relu/bias + mask -> a1h interior ----
                for c in range(NCH):
                    p1 = pbank(c % 8, f"p1_{b}_{c}")
                    co = CH * (c % SC)  # offset within the superchunk-half
                    for g in range(G):
                        nc.tensor.matmul(
                            p1[32 * g : 32 * (g + 1), :],
                            w1sb[:, g, :],
                            xg[(g, c // SC)][:, co : co + CH],
                            start=True,
                            stop=True,
                            tile_position=(0, 32 * g),
                        )
                    u1 = upool.tile([128, CH], F32, name=f"u1_{b}_{c}", tag="u1")
                    nc.scalar.activation(
                        u1, p1, mybir.ActivationFunctionType.Relu, bias=b1sb[:, 0:1]
                    )
                    nc.vector.scalar_tensor_tensor(
                        out=a1h[:, 1 + R * c : 1 + R * (c + 1), 1 : 1 + W],
                        in0=u1.rearrange("q (a w) -> q a w", w=W),
                        scalar=0.0,
                        in1=mM[:, CH * c : CH * (c + 1)].rearrange(
                            "q (a w) -> q a w", w=W
                        ),
                        op0=mybir.AluOpType.add,
                        op1=mybir.AluOpType.mult,
                    )

                # chunk-scrambled: mS[32j+c, g, k, p] <- mup[b, g, (4k+j)*CH + p]
                # issued after stage A so x loads win the SDMA early window
                mS = mpool.tile([128, G * NSC * CH], BF16, name=f"mS_{b}", tag="mS")
                stgS = mpool.tile([128, G * NSC * CH], BF16, name=f"stgS_{b}", tag="stgS")
                nc.gpsimd.memset(stgS, 0.0)
                for j in range(SC):
                    nc.gpsimd.dma_start(
                        out=stgS[32 * j : 32 * j + 1, :], in_=mupS[b, j : j + 1, :]
                    )
                nc.vector.stream_shuffle(mS, stgS, [0] * 32)

                # ---- stages B+C interleaved per superchunk ----
                for k in range(NSC):
                    # -- B: conv2 (16-tile) + relu/bias + mask -> a2s --
                    p2 = [pbank(4 + g, f"p2_{b}_{k}_{g}") for g in range(G)]
                    for t in range(9):
                        ky, kx = divmod(t, 3)
                        for g in range(G):
                            for j in range(SC):
                                c = SC * k + j
                                nc.tensor.matmul(
                                    p2[g][32 * j : 32 * (j + 1), :],
                                    w2sb[32 * g : 32 * (g + 1), t, :],
                                    a1h[
                                        32 * g : 32 * (g + 1),
                                        R * c + ky : R * c + ky + R,
                                        kx : kx + W,
                                    ],
                                    start=(t == 0),
                                    stop=(t == 8),
                                    tile_position=(32 * g, 32 * j),
                                    skip_group_check=True,
                                )
                    a2s = {}
                    for g in range(G):
                        u2 = upool.tile([128, CH], F32, name=f"u2_{b}_{k}_{g}", tag="u2")
                        nc.scalar.activation(
                            u2,
                            p2[g],
                            mybir.ActivationFunctionType.Relu,
                            bias=b2sb[:, g : g + 1],
                        )
                        at = a2pool.tile([128, CH], F32, name=f"a2_{b}_{k}_{g}", tag="a2s")
                        nc.gpsimd.tensor_mul(at, u2, mS[:, (g * NSC + k) * CH : (g * NSC + k + 1) * CH])
                        a2s[g] = at

                    # -- C: conv3 (row-tiled, 8 banks) + residual + relu -> out --
                    for g in range(G):
                        ot = opool.tile([128, SCW], F32, name=f"o_{b}_{k}_{g}", tag="o")
                        for j in range(SC):
                            p3 = pbank(4 * (g % 2) + j, f"p3_{b}_{k}_{g}_{j}")
                            nc.tensor.matmul(
                                p3,
                                w3sb[32 * j : 32 * (j + 1), g, :],
                                a2s[g][32 * j : 32 * (j + 1), :],
                                start=True,
                                stop=True,
                                tile_position=(32 * j, 0),
                            )
                            # pre-relu value (p3 + bias3 + x) straight into ot
                            nc.vector.scalar_tensor_tensor(
                                out=ot[:, CH * j : CH * (j + 1)],
                                in0=p3,
                                scalar=b3sb[:, g : g + 1],
                                in1=xg[(g, k)][:, CH * j : CH * (j + 1)],
                                op0=mybir.AluOpType.add,
                                op1=mybir.AluOpType.add,
                            )
                        # one wide in-place relu per (g, superchunk)
                        nc.scalar.activation(
                            ot, ot, mybir.ActivationFunctionType.Relu
                        )
                        nc.scalar.dma_start(
                            out=ys[b, 128 * g : 128 * (g + 1), SCW * k : SCW * (k + 1)],
                            in_=ot,
                        )

    nc.finalize()
    return nc


def pack_params(w1, g1, b1, m1, v1, w2, g2, b2, m2, v2, w3, g3, b3, m3, v3):
    """Fold BN into weights/biases and lay out for the PE mappings."""
    f32 = np.float32
    s1 = (g1 / np.sqrt(v1 + EPS)).astype(f32)
    s2 = (g2 / np.sqrt(v2 + EPS)).astype(f32)
    s3 = (g3 / np.sqrt(v3 + EPS)).astype(f32)
    c1 = (b1 - m1 * s1).astype(f32)
    c2 = (b2 - m2 * s2).astype(f32)
    c3 = (b3 - m3 * s3).astype(f32)

    w1q = w1[:, :, 0, 0].astype(f32)  # [128 out, 128 in-per-group]
    w3q = w3[:, :, 0, 0].astype(f32)  # [512 out, 32 in-per-group]

    w1l = np.zeros([128, G, 32], f32)
    for g in range(G):
        blk = w1q[32 * g : 32 * (g + 1), :] * s1[32 * g : 32 * (g + 1), None]
        w1l[:, g, :] = blk.T  # [ci=128, co=32]

    w2l = np.zeros([128, 9, 32], f32)
    for g in range(G):
        sg = s2[32 * g : 32 * (g + 1), None]
        for t in range(9):
            ky, kx = divmod(t, 3)
            blk = w2[32 * g : 32 * (g + 1), :, ky, kx].astype(f32) * sg
            w2l[32 * g : 32 * (g + 1), t, :] = blk.T  # [ci=32, co=32]

    w3l = np.zeros([128, G, 128], f32)
    for g in range(G):
        blk = (w3q[128 * g : 128 * (g + 1), :] * s3[128 * g : 128 * (g + 1), None]).T
        for j in range(4):
            w3l[32 * j : 32 * (j + 1), g, :] = blk  # [ci=32, co=128], j-replicated

    b1v = c1.reshape(128, 1).astype(f32)
    b2v = np.zeros([128, G], f32)
    for g in range(G):
        for j in range(4):
            b2v[32 * j : 32 * (j + 1), g] = c2[32 * g : 32 * (g + 1)]
    b3v = c3.reshape(G, 128).T.astype(f32).copy()
    return dict(w1l=w1l, w2l=w2l, w3l=w3l, b1d=b1v, b2d=b2v, b3d=b3v)


def upsample_mask(mask):
    """[16, 4, 7, 7] -> bf16 ([16,4,3136] channel-major, [16,4,4*2*392] scrambled).

    mupS[b, j, g, k, p] = m[b, g, (4k+j)*CH + p] (conv2/3's chunk-scrambled view)."""
    import ml_dtypes
    m = np.repeat(np.repeat(mask, H // 7, axis=2), W // 7, axis=3)
    m = np.ascontiguousarray(m.reshape(mask.shape[0], G, PIX))
    mc = m.reshape(mask.shape[0], G, NSC, SC, CH)  # [b, g, k, j, p]
    ms = np.ascontiguousarray(mc.transpose(0, 3, 1, 2, 4))  # [b, j, g, k, p]
    ms = ms.reshape(mask.shape[0], SC, G * NSC * CH)
    return m.astype(ml_dtypes.bfloat16), ms.astype(ml_dtypes.bfloat16)


def _run(inputs, **spmd_kwargs):
    x = np.asarray(inputs["x"], dtype=np.float32)
    mask = np.asarray(inputs["mask"], dtype=np.float32)
    params = pack_params(
        *(np.asarray(inputs[k], dtype=np.float32)
          for k in ("w1", "g1", "b1", "m1", "v1",
                    "w2", "g2", "b2", "m2", "v2",
                    "w3", "g3", "b3", "m3", "v3"))
    )
    mup, mupS = upsample_mask(mask)
    xr = np.ascontiguousarray(x.reshape(B_TOT, CIN, PIX))

    nc = build_nc()
    in_maps = []
    for c in range(N_CORES):
        sl = slice(B * c, B * (c + 1))
        m = {
            "xs": np.ascontiguousarray(xr[sl]),
            "mup": np.ascontiguousarray(mup[sl]),
            "mupS": np.ascontiguousarray(mupS[sl]),
        }
        m.update(params)
        in_maps.append(m)

    res = run_bass_kernel_spmd(nc, in_maps, core_ids=list(range(N_CORES)), **spmd_kwargs)
    out = np.concatenate([r["ys"] for r in res.results], axis=0)
    return out.reshape(B_TOT, CIN, H, W), res


def kernel(**inputs):
    out, _ = _run(inputs)
    return out


if __name__ == "__main__":
    # smoke: build only
    nc = build_nc()
    print("built ok")



# revision 2
# speedup vs baseline: 1.0741x; 1.0741x over previous
"""Trainium2 Bass kernel for nn_Bottleneck_refine (grouped bottleneck + block mask).

Reference computation (per image b):
    m   = upsample(mask[b])            # [4,7,7] -> per-group 56x56 {0,1}
    t1  = conv1x1_g4(x * m1)           # 512 -> 128, but 1x1 commutes with mask
    a1  = m . relu(s1*t1 + c1)
    t2  = conv3x3_g4(a1)               # 128 -> 128 (pad 1)
    a2  = m . relu(s2*t2 + c2)
    y   = relu(s3*conv1x1_g4(a2) + c3 + x)

Identity used: for m in {0,1}:  m*relu(z) == relu(m*z), and the 1x1 conv
commutes with per-pixel masking, so the input mask multiply is absorbed.

Sharding: data-parallel over batch, 2 images per core on 8 cores.
All I/O and activations are bf16 (host converts): per-core HBM traffic
~= 6.4 MB in + 6.4 MB out -> memory roofline ~36us at 358 GB/s.

Layouts per image (all SBUF, bf16, [partition, free]):
  x_g     [128, 3136]  per group g (channel-major, pixel row-major)
  a1h     [128, 58*58] halo'd masked mid activation (channels 4g x 32)
  a2s     [128, 392] per (g, superchunk): partition 32j+co = chunk 4k+j, ch co
  chunks: 7 image rows (392 px), 8 chunks, 2 superchunks of 4 chunks.

PE mapping:
  conv1: 128x32 column tiling, tile (0, 32g), one PSUM bank, channel-major out.
  conv2: 32x32 16-tile packing, tile (32g, 32j): row=group, col=chunk-in-sc.
         9 taps accumulate into bank g; output chunk-scrambled.
  conv3: 32x128 row tiling, tile (32j, 0) per (group, chunk), full-width out.

DMA plan: all x loads (both images) on the sync HWDGE ring up front, all
output stores on the scalar HWDGE ring, masks on the gpsimd SWDGE ring.
Final relu is split across vector/scalar/gpsimd engines.
"""

import numpy as np

import concourse.bass as bass
import concourse.tile as tile
from concourse import bacc, mybir
from concourse.bass_utils import run_bass_kernel_spmd

F32 = mybir.dt.float32
BF16 = mybir.dt.bfloat16
EPS = 1e-5

N_CORES = 8
B_TOT = 16
B = B_TOT // N_CORES  # images per core
G = 4
CIN = 512
MID = 128
H = W = 56
PIX = H * W  # 3136
R = 7  # image rows per chunk
CH = R * W  # 392 pixels per chunk
NCH = H // R  # 8 chunks
SC = 4  # chunks per superchunk
NSC = NCH // SC  # 2 superchunks
HH = H + 2  # halo'd height/width (58)
SCW = SC * CH  # pixels per superchunk (1568)

# final relu split across engines (vector / scalar / gpsimd), sums to SCW
RELU_V = 960
RELU_A = 352
RELU_G = SCW - RELU_V - RELU_A


def build_nc():
    # Bacc (not Bass): its compile()/finalize() pipeline legalizes sync waits
    # (>=2 waits per instruction are split into EventSemaphore instructions,
    # which this walrus build requires) and moves matmul waits to ldweights.
    nc = bacc.Bacc(None, target_bir_lowering=False)

    xs = nc.dram_tensor("xs", [B, G, 128, PIX], BF16, kind="ExternalInput")
    mup = nc.dram_tensor("mup", [B, G, PIX], BF16, kind="ExternalInput")
    mupS = nc.dram_tensor("mupS", [B, SC, G * NSC * CH], BF16, kind="ExternalInput")
    w1l = nc.dram_tensor("w1l", [128, G, 32], BF16, kind="ExternalInput")
    w2l = nc.dram_tensor("w2l", [128, 9, 32], BF16, kind="ExternalInput")
    w3l = nc.dram_tensor("w3l", [128, G, 128], BF16, kind="ExternalInput")
    b1d = nc.dram_tensor("b1d", [128, 1], F32, kind="ExternalInput")
    b2d = nc.dram_tensor("b2d", [128, G], F32, kind="ExternalInput")
    b3d = nc.dram_tensor("b3d", [128, G], F32, kind="ExternalInput")
    ys = nc.dram_tensor("ys", [B, G, 128, PIX], BF16, kind="ExternalOutput")

    with tile.TileContext(nc) as tc:
        with (
            tc.tile_pool(name="consts", bufs=1) as consts,
            tc.tile_pool(name="xpool", bufs=2 * G * NSC) as xpool,
            tc.tile_pool(name="mpool", bufs=2) as mpool,
            tc.tile_pool(name="a1pool", bufs=2) as a1pool,
            tc.tile_pool(name="a2pool", bufs=8) as a2pool,
            tc.tile_pool(name="upool", bufs=4) as upool,
            tc.tile_pool(name="opool", bufs=5) as opool,
            tc.tile_pool(name="psum", bufs=1, space="PSUM") as psum,
        ):
            # ---- constants (loaded once) ----
            w1sb = consts.tile([128, G, 32], BF16)
            w2sb = consts.tile([128, 9, 32], BF16)
            w3sb = consts.tile([128, G, 128], BF16)
            b1sb = consts.tile([128, 1], F32)
            b2sb = consts.tile([128, G], F32)
            b3sb = consts.tile([128, G], F32)
            nc.sync.dma_start(out=w1sb, in_=w1l[:])
            nc.sync.dma_start(out=w2sb, in_=w2l[:])
            nc.sync.dma_start(out=w3sb, in_=w3l[:])
            nc.sync.dma_start(out=b1sb, in_=b1d[:])
            nc.sync.dma_start(out=b2sb, in_=b2d[:])
            nc.sync.dma_start(out=b3sb, in_=b3d[:])

            # ---- load x for BOTH images up front (sync ring only, so the
            # output stores on the scalar ring can never block input flow)
            xg = {}
            for b in range(B):
                for k in range(NSC):
                    for g in range(G):
                        xt = xpool.tile(
                            [128, SCW], BF16, name=f"x_{b}_{g}_{k}", tag="x"
                        )
                        nc.sync.dma_start(
                            out=xt,
                            in_=xs[b, g, :, SCW * k : SCW * (k + 1)],
                        )
                        xg[(b, g, k)] = xt

            # PSUM bank plan (8 banks, one tag per bank, bufs=1 each):
            #   stage A (conv1): banks 0-7 rotate per chunk
            #   stage B (conv2): banks 4-7 held per superchunk (bank = group)
            #   stage C (conv3): banks 0-3 / 4-7 (bank = row tile j), by g parity
            def pbank(i, name):
                return psum.tile([128, 512], F32, name=name, tag=f"bk{i}")[:, :CH]

            # PE warmup: keep TensorE busy during the input-DMA head so the
            # HAM clock gate reaches 8/8 before conv1; results are discarded.
            warm = psum.tile([128, 512], F32, name="warm", tag="bk7")[:, :128]
            for wi in range(24):
                nc.tensor.matmul(
                    warm[0:32, :],
                    w1sb[:, 0, :],
                    w3sb[:, 0, :],
                    start=True,
                    stop=True,
                    tile_position=(0, 0),
                )

            for b in range(B):
                # ---- masks via broadcast DMA (partition step-0) ----
                # channel-major: partition 32g+c <- mup[b, g, :]; bf16
                mM = mpool.tile([128, PIX], BF16, name=f"mM_{b}", tag="mM")
                stgM = mpool.tile([128, PIX], BF16, name=f"stgM_{b}", tag="stgM")
                nc.gpsimd.memset(stgM, 0.0)
                for g in range(G):
                    nc.gpsimd.dma_start(
                        out=stgM[32 * g : 32 * g + 1, :], in_=mup[b, g : g + 1, :]
                    )
                nc.vector.stream_shuffle(mM, stgM, [0] * 32)

                # ---- halo'd a1: zero only the 1px border (interior is
                # fully written by stage A)
                a1h = a1pool.tile([128, HH, HH], BF16, name=f"a1h_{b}", tag="a1h")
                nc.gpsimd.memset(a1h[:, 0, :], 0.0)
                nc.gpsimd.memset(a1h[:, HH - 1, :], 0.0)
                nc.gpsimd.memset(a1h[:, 1 : HH - 1, 0:1], 0.0)
                nc.gpsimd.memset(a1h[:, 1 : HH - 1, HH - 1 : HH], 0.0)

                # ---- stage A: conv1 + relu/bias + mask -> a1h interior ----
                for c in range(NCH):
                    p1 = pbank(c % 8, f"p1_{b}_{c}")
                    co = CH * (c % SC)  # offset within the superchunk-half
                    for g in range(G):
                        nc.tensor.matmul(
                            p1[32 * g : 32 * (g + 1), :],
                            w1sb[:, g, :],
                            xg[(b, g, c // SC)][:, co : co + CH],
                            start=True,
                            stop=True,
                            tile_position=(0, 32 * g),
                        )
                    u1 = upool.tile([128, CH], BF16, name=f"u1_{b}_{c}", tag="u1")
                    nc.scalar.activation(
                        u1, p1, mybir.ActivationFunctionType.Relu, bias=b1sb[:, 0:1]
                    )
                    nc.vector.scalar_tensor_tensor(
                        out=a1h[:, 1 + R * c : 1 + R * (c + 1), 1 : 1 + W],
                        in0=u1.rearrange("q (a w) -> q a w", w=W),
                        scalar=0.0,
                        in1=mM[:, CH * c : CH * (c + 1)].rearrange(
                            "q (a w) -> q a w", w=W
                        ),
                        op0=mybir.AluOpType.add,
                        op1=mybir.AluOpType.mult,
                    )

                # chunk-scrambled: mS[32j+c, g, k, p] <- mup[b, g, (4k+j)*CH + p]
                # issued after stage A so x loads win the SDMA early window
                mS = mpool.tile([128, G * NSC * CH], BF16, name=f"mS_{b}", tag="mS")
                stgS = mpool.tile(
                    [128, G * NSC * CH], BF16, name=f"stgS_{b}", tag="stgS"
                )
                nc.gpsimd.memset(stgS, 0.0)
                for j in range(SC):
                    nc.gpsimd.dma_start(
                        out=stgS[32 * j : 32 * j + 1, :], in_=mupS[b, j : j + 1, :]
                    )
                nc.vector.stream_shuffle(mS, stgS, [0] * 32)

                # ---- stages B+C interleaved per superchunk ----
                for k in range(NSC):
                    # -- B: conv2 (16-tile) + relu/bias + mask -> a2s --
                    p2 = [pbank(4 + g, f"p2_{b}_{k}_{g}") for g in range(G)]
                    for t in range(9):
                        ky, kx = divmod(t, 3)
                        for g in range(G):
                            for j in range(SC):
                                c = SC * k + j
                                nc.tensor.matmul(
                                    p2[g][32 * j : 32 * (j + 1), :],
                                    w2sb[32 * g : 32 * (g + 1), t, :],
                                    a1h[
                                        32 * g : 32 * (g + 1),
                                        R * c + ky : R * c + ky + R,
                                        kx : kx + W,
                                    ],
                                    start=(t == 0),
                                    stop=(t == 8),
                                    tile_position=(32 * g, 32 * j),
                                    skip_group_check=True,
                                )
                    a2s = {}
                    for g in range(G):
                        u2 = upool.tile([128, CH], BF16, name=f"u2_{b}_{k}_{g}", tag="u2")
                        nc.scalar.activation(
                            u2,
                            p2[g],
                            mybir.ActivationFunctionType.Relu,
                            bias=b2sb[:, g : g + 1],
                        )
                        at = a2pool.tile([128, CH], BF16, name=f"a2_{b}_{k}_{g}", tag="a2s")
                        nc.gpsimd.tensor_mul(
                            at, u2, mS[:, (g * NSC + k) * CH : (g * NSC + k + 1) * CH]
                        )
                        a2s[g] = at

                    # -- C: conv3 (row-tiled) + bias + residual -> ot, then
                    # relu split across vector/scalar/gpsimd, store on scalar
                    for g in range(G):
                        ot = opool.tile([128, SCW], BF16, name=f"o_{b}_{k}_{g}", tag="o")
                        for j in range(SC):
                            p3 = pbank(4 * (g % 2) + j, f"p3_{b}_{k}_{g}_{j}")
                            nc.tensor.matmul(
                                p3,
                                w3sb[32 * j : 32 * (j + 1), g, :],
                                a2s[g][32 * j : 32 * (j + 1), :],
                                start=True,
                                stop=True,
                                tile_position=(32 * j, 0),
                            )
                            # pre-relu value (p3 + bias3 + x) straight into ot
                            nc.vector.scalar_tensor_tensor(
                                out=ot[:, CH * j : CH * (j + 1)],
                                in0=p3,
                                scalar=b3sb[:, g : g + 1],
                                in1=xg[(b, g, k)][:, CH * j : CH * (j + 1)],
                                op0=mybir.AluOpType.add,
                                op1=mybir.AluOpType.add,
                            )
                        # relu split across three engines
                        nc.vector.tensor_relu(ot[:, :RELU_V], ot[:, :RELU_V])
                        nc.scalar.activation(
                            ot[:, RELU_V : RELU_V + RELU_A],
                            ot[:, RELU_V : RELU_V + RELU_A],
                            mybir.ActivationFunctionType.Relu,
                        )
                        nc.gpsimd.tensor_relu(
                            ot[:, RELU_V + RELU_A :], ot[:, RELU_V + RELU_A :]
                        )
                        nc.scalar.dma_start(
                            out=ys[b, g, :, SCW * k : SCW * (k + 1)],
                            in_=ot,
                        )

    nc.finalize()
    return nc


def pack_params(w1, g1, b1, m1, v1, w2, g2, b2, m2, v2, w3, g3, b3, m3, v3):
    """Fold BN into weights/biases and lay out for the PE mappings."""
    import ml_dtypes

    f32 = np.float32
    bf16 = ml_dtypes.bfloat16
    s1 = (g1 / np.sqrt(v1 + EPS)).astype(f32)
    s2 = (g2 / np.sqrt(v2 + EPS)).astype(f32)
    s3 = (g3 / np.sqrt(v3 + EPS)).astype(f32)
    c1 = (b1 - m1 * s1).astype(f32)
    c2 = (b2 - m2 * s2).astype(f32)
    c3 = (b3 - m3 * s3).astype(f32)

    w1q = w1[:, :, 0, 0].astype(f32)  # [128 out, 128 in-per-group]
    w3q = w3[:, :, 0, 0].astype(f32)  # [512 out, 32 in-per-group]

    w1l = np.zeros([128, G, 32], f32)
    for g in range(G):
        blk = w1q[32 * g : 32 * (g + 1), :] * s1[32 * g : 32 * (g + 1), None]
        w1l[:, g, :] = blk.T  # [ci=128, co=32]

    w2l = np.zeros([128, 9, 32], f32)
    for g in range(G):
        sg = s2[32 * g : 32 * (g + 1), None]
        for t in range(9):
            ky, kx = divmod(t, 3)
            blk = w2[32 * g : 32 * (g + 1), :, ky, kx].astype(f32) * sg
            w2l[32 * g : 32 * (g + 1), t, :] = blk.T  # [ci=32, co=32]

    w3l = np.zeros([128, G, 128], f32)
    for g in range(G):
        blk = (w3q[128 * g : 128 * (g + 1), :] * s3[128 * g : 128 * (g + 1), None]).T
        for j in range(4):
            w3l[32 * j : 32 * (j + 1), g, :] = blk  # [ci=32, co=128], j-replicated

    b1v = c1.reshape(128, 1).astype(f32)
    b2v = np.zeros([128, G], f32)
    for g in range(G):
        for j in range(4):
            b2v[32 * j : 32 * (j + 1), g] = c2[32 * g : 32 * (g + 1)]
    b3v = c3.reshape(G, 128).T.astype(f32).copy()
    return dict(
        w1l=w1l.astype(bf16),
        w2l=w2l.astype(bf16),
        w3l=w3l.astype(bf16),
        b1d=b1v,
        b2d=b2v,
        b3d=b3v,
    )


def upsample_mask(mask):
    """[16, 4, 7, 7] -> bf16 ([16,4,3136] channel-major, [16,4,4*2*392] scrambled).

    mupS[b, j, g, k, p] = m[b, g, (4k+j)*CH + p] (conv2/3's chunk-scrambled view)."""
    import ml_dtypes

    m = np.repeat(np.repeat(mask, H // 7, axis=2), W // 7, axis=3)
    m = np.ascontiguousarray(m.reshape(mask.shape[0], G, PIX))
    mc = m.reshape(mask.shape[0], G, NSC, SC, CH)  # [b, g, k, j, p]
    ms = np.ascontiguousarray(mc.transpose(0, 3, 1, 2, 4))  # [b, j, g, k, p]
    ms = ms.reshape(mask.shape[0], SC, G * NSC * CH)
    return m.astype(ml_dtypes.bfloat16), ms.astype(ml_dtypes.bfloat16)


def _run(inputs, **spmd_kwargs):
    import ml_dtypes

    x = np.asarray(inputs["x"], dtype=np.float32)
    mask = np.asarray(inputs["mask"], dtype=np.float32)
    params = pack_params(
        *(np.asarray(inputs[k], dtype=np.float32)
          for k in ("w1", "g1", "b1", "m1", "v1",
                    "w2", "g2", "b2", "m2", "v2",
                    "w3", "g3", "b3", "m3", "v3"))
    )
    mup, mupS = upsample_mask(mask)
    # [B_TOT, G, 128, PIX] bf16, channel-major within group
    xr = np.ascontiguousarray(
        x.reshape(B_TOT, G, 128, PIX).astype(ml_dtypes.bfloat16)
    )

    nc = build_nc()
    in_maps = []
    for c in range(N_CORES):
        sl = slice(B * c, B * (c + 1))
        m = {
            "xs": np.ascontiguousarray(xr[sl]),
            "mup": np.ascontiguousarray(mup[sl]),
            "mupS": np.ascontiguousarray(mupS[sl]),
        }
        m.update(params)
        in_maps.append(m)

    res = run_bass_kernel_spmd(nc, in_maps, core_ids=list(range(N_CORES)), **spmd_kwargs)
    out = np.concatenate([r["ys"] for r in res.results], axis=0)
    return out.astype(np.float32).reshape(B_TOT, CIN, H, W), res


def kernel(**inputs):
    out, _ = _run(inputs)
    return out


if __name__ == "__main__":
    # smoke: build only
    nc = build_nc()
    print("built ok")
